# revision 15
# baseline (speedup 1.0000x reference)
"""STEBitLinear Trainium2 kernel.

y[b,s,o] = sum_i x[b,s,i] * sign(w[o,i]) * scale[o, i//128]

Strategy: data-parallel over the flattened (b,s) dim across 8 NeuronCores
(weights/scales replicated, no collectives). Per core (M=1024, K=N=4096):
  - x shard cast to bf16, transposed into a resident SBUF x^T via PE
    matmuls against identity (cheap: 3% of PE time, and it runs during
    the load-bound prologue)
  - per 512-wide out-feature tile: w_eff = sign*scale built in bf16 with
    per-partition tensor_scalar ops split 3:1 between DVE and ACT, then
    transposed on the PE against identity; the PSUM->SBUF evacuations
    alternate between DVE and ACT so neither engine saturates (the
    baseline put all of them on DVE, which was the second bottleneck)
  - w^T build chunks are emitted interleaved into the consuming m-loop so
    the in-order DVE queue never delays accumulator evacuations
  - PE main matmuls: 128x128x512 bf16, K accumulated in PSUM
  - PSUM evacuated to bf16 by DVE, stored as bf16, host casts to fp32
  (A DMA-xbar-transpose variant was tried and is numerically correct with
  a drain-based sync, but the xbar ucode's completion semaphores stall
  ~19us per transfer on HW, so the PE transpose wins.)

DMA queues: w loads ride the gpsimd (SWDGE) queue, x loads + y stores the
sync queue, xbar transposes the scalar queue.
"""

import sys

for _p in ("/opt/trn_rl_repo", "/opt/pypackages"):
    if _p not in sys.path:
        sys.path.append(_p)

import numpy as np

import concourse.bacc as bacc
import concourse.mybir as mybir
from concourse.bass import _add_dep_helper
from concourse.bass_utils import run_bass_kernel_spmd
from concourse.masks import make_identity
from concourse.tile import TileContext

N_CORES = 8
B, S, IN_F, OUT_F = 4, 2048, 4096, 4096
GROUP = 128
M_FULL = B * S  # 8192


def build_program(M=M_FULL // N_CORES, K=IN_F, N=OUT_F, n_tile=512, ld=1024):
    """Emit the per-core Bass program (SPMD: same program on all cores)."""
    P = 128
    KT = K // P            # k chunks of 128 (contraction)
    MT = M // P            # m tiles of 128
    NT = N // n_tile       # out-feature tiles
    NSUB = n_tile // P     # 128-row o sub-blocks per o tile
    LC = K // ld           # load chunks per 128-row block
    LG = ld // P           # 128-col groups per load chunk
    G = K // GROUP         # scale groups along in_features
    NB = N // P            # o blocks of 128
    bf16 = mybir.dt.bfloat16
    f32 = mybir.dt.float32

    nc = bacc.Bacc("TRN2", target_bir_lowering=False, debug=False)
    x_d = nc.dram_tensor("x", [M, K], f32, kind="ExternalInput").ap()
    w_d = nc.dram_tensor("sw", [N, K], f32, kind="ExternalInput").ap()
    sc_d = nc.dram_tensor("sc", [N, G], f32, kind="ExternalInput").ap()
    y_d = nc.dram_tensor("y", [M, N], bf16, kind="ExternalOutput").ap()

    with TileContext(nc) as tc:
        with (
            tc.tile_pool(name="consts", bufs=1) as consts,
            tc.tile_pool(name="xt_pool", bufs=1) as xt_pool,
            tc.tile_pool(name="wt_pool", bufs=3) as wt_pool,
            tc.tile_pool(name="wload", bufs=3) as wload_pool,
            tc.tile_pool(name="wstage", bufs=6) as wstage_pool,
            tc.tile_pool(name="xload", bufs=2) as xload_pool,
            tc.tile_pool(name="xstage", bufs=2) as xstage_pool,
            tc.tile_pool(name="ysb", bufs=4) as y_pool,
            tc.tile_pool(name="acc", bufs=5, space="PSUM") as psum_a,
            tc.tile_pool(name="ptr", bufs=3, space="PSUM") as psum_t,
        ):
            ident = consts.tile([P, P], bf16)
            make_identity(nc, ident)

            # scales resident: sc_sb[p, ob*G + g] = scales[ob*128 + p, g]
            sc_sb = consts.tile([P, NB * G], f32)
            nc.sync.dma_start(
                out=sc_sb.rearrange("p (ob g) -> p ob g", ob=NB),
                in_=sc_d.rearrange("(ob p) g -> p ob g", p=P),
            )

            # work splitters: scale ops 3:1 DVE:ACT, casts/evacs alternate
            st = [0, 0]

            def veng_scale(out, in_, scalar):
                i = st[0]
                st[0] += 1
                if i % 4 < 3:
                    nc.vector.tensor_scalar_mul(out=out, in0=in_, scalar1=scalar)
                else:
                    nc.scalar.mul(out, in_, scalar)

            def veng_copy(out, in_):
                i = st[1]
                st[1] += 1
                if i % 2 == 0:
                    nc.vector.tensor_copy(out=out, in_=in_)
                else:
                    nc.scalar.copy(out=out, in_=in_)

            # ---- x^T resident (bf16): xT[p, k, m] ----
            xT = xt_pool.tile([P, KT, M], bf16)

            def emit_x_chunk(mt, lc):
                xin = xload_pool.tile([P, ld], f32, tag="xload")
                nc.sync.dma_start(
                    out=xin,
                    in_=x_d[mt * P:(mt + 1) * P, lc * ld:(lc + 1) * ld],
                )
                xst = xstage_pool.tile([P, ld], bf16, tag="xstage")
                veng_copy(xst, xin)
                # PE transpose against identity, 4 blocks per PSUM tile
                for h in range(LG // 4):
                    pt = psum_t.tile([P, 512], f32, tag="pt")
                    for g in range(4):
                        c = h * 4 + g
                        nc.tensor.matmul(
                            pt[:, g * P:(g + 1) * P],
                            xst[:, c * P:(c + 1) * P],
                            ident,
                            start=True,
                            stop=True,
                        )
                    pt_v = pt.rearrange("p (g c) -> p g c", g=4)
                    veng_copy(
                        xT[:, lc * LG + h * 4:lc * LG + h * 4 + 4,
                           mt * P:(mt + 1) * P],
                        pt_v,
                    )

            # ---- w_eff^T build for one 512-wide o tile (PE transposes) ----
            def emit_build_chunk(wT, ot, c):
                """One (j, lc) chunk of the wT build: load, scale, transpose."""
                j, lc = divmod(c, LC)
                ob = ot * NSUB + j
                win = wload_pool.tile([P, ld], f32, tag="wload")
                nc.gpsimd.dma_start(
                    out=win,
                    in_=w_d[ob * P:(ob + 1) * P, lc * ld:(lc + 1) * ld],
                )
                wst = wstage_pool.tile([P, ld], bf16, tag="wstage")
                for g in range(LG):
                    gk = lc * LG + g
                    veng_scale(
                        wst[:, g * P:(g + 1) * P],
                        win[:, g * P:(g + 1) * P],
                        sc_sb[:, ob * G + gk:ob * G + gk + 1],
                    )
                nc.scalar.dma_start(
                    out=wT[:, lc * LG:(lc + 1) * LG, j * P:(j + 1) * P],
                    in_=wst,
                    transpose=True,
                )

            NCH = NSUB * LC  # build chunks per o tile (16)

            def build_wT_all(ot):
                wT = wt_pool.tile([P, KT, n_tile], bf16, tag="wt")
                for c in range(NCH):
                    emit_build_chunk(wT, ot, c)
                dr = nc.scalar.drain()
                return wT, dr

            # ---- prologue: first two wT builds, then x phase ----
            wT_cur, dr_cur = build_wT_all(0)
            wT_nxt, dr_nxt = (build_wT_all(1) if NT > 1 else (None, None))

            # x phase (PE transposes interleave with the load-bound stream)
            for mt in range(MT):
                for lc in range(LC):
                    emit_x_chunk(mt, lc)

            # ---- main loop over o tiles, build for ot+2 interleaved so
            #      DVE evacs never sit behind a burst of scale ops ----
            for ot in range(NT):
                build = ot + 2 < NT
                wT_pre = (wt_pool.tile([P, KT, n_tile], bf16, tag="wt",
                                       name="wT_pre")
                          if build else None)
                for mt in range(MT):
                    acc = psum_a.tile([P, n_tile], f32, tag="acc")
                    for k in range(KT):
                        mm = nc.tensor.matmul(
                            acc,
                            xT[:, k, mt * P:(mt + 1) * P],
                            wT_cur[:, k],
                            start=(k == 0),
                            stop=(k == KT - 1),
                        )
                        if mt == 0 and k == 0:
                            _add_dep_helper(mm.ins, dr_cur.ins, sync=True,
                                            reason="wT xbar drain")
                    ysb = y_pool.tile([P, n_tile], bf16, tag="ysb")
                    nc.vector.tensor_copy(out=ysb, in_=acc)
                    nc.sync.dma_start(
                        out=y_d[mt * P:(mt + 1) * P,
                                ot * n_tile:(ot + 1) * n_tile],
                        in_=ysb,
                    )
                    if build:
                        for c in range(2 * mt, 2 * mt + 2):
                            emit_build_chunk(wT_pre, ot + 2, c)
                dr_pre = nc.scalar.drain() if build else None
                wT_cur, dr_cur = wT_nxt, dr_nxt
                wT_nxt, dr_nxt = wT_pre, dr_pre

    nc.compile()
    return nc


_nc_cache = {}


def _get_nc(key, **kw):
    if key not in _nc_cache:
        _nc_cache[key] = build_program(**kw)
    return _nc_cache[key]


def _make_in_maps(x, sign_weights, scales):
    M_SH = M_FULL // N_CORES
    xf = np.ascontiguousarray(x.reshape(M_FULL, IN_F).astype(np.float32, copy=False))
    sw = np.ascontiguousarray(sign_weights.astype(np.float32, copy=False))
    sc = np.ascontiguousarray(scales.reshape(OUT_F, IN_F // GROUP))
    return [
        {"x": xf[c * M_SH:(c + 1) * M_SH], "sw": sw, "sc": sc}
        for c in range(N_CORES)
    ]


def _assemble(results):
    y = np.concatenate([results[c]["y"] for c in range(N_CORES)], axis=0)
    return y.reshape(B, S, OUT_F).astype(np.float32)


def kernel(x: np.ndarray, sign_weights: np.ndarray, scales: np.ndarray) -> np.ndarray:
    nc = _get_nc("full")
    in_maps = _make_in_maps(x, sign_weights, scales)
    res = run_bass_kernel_spmd(nc, in_maps, core_ids=list(range(N_CORES)))
    return _assemble(res.results)


# revision 17
# speedup vs baseline: 1.6578x; 1.6578x over previous
"""STEBitLinear Trainium2 kernel.

y[b,s,o] = sum_i x[b,s,i] * sign(w[o,i]) * scale[o, i//128]

Strategy: data-parallel over the flattened (b,s) dim across 8 NeuronCores
(weights/scales replicated, no collectives). Per core (M=1024, K=N=4096):
  - x shard cast to bf16, transposed into a resident SBUF x^T via PE
    matmuls against identity (cheap: 3% of PE time, and it runs during
    the load-bound prologue)
  - per 512-wide out-feature tile: w_eff = sign*scale built in bf16 with
    per-partition tensor_scalar ops split 3:1 between DVE and ACT, then
    transposed on the PE against identity; the PSUM->SBUF evacuations
    alternate between DVE and ACT so neither engine saturates (the
    baseline put all of them on DVE, which was the second bottleneck)
  - w^T build chunks are emitted interleaved into the consuming m-loop so
    the in-order DVE queue never delays accumulator evacuations
  - PE main matmuls: 128x128x512 bf16, K accumulated in PSUM
  - PSUM evacuated to bf16 by DVE, stored as bf16, host casts to fp32
  (A DMA-xbar-transpose variant was tried and is numerically correct with
  a drain-based sync, but the xbar ucode's completion semaphores stall
  ~19us per transfer on HW, so the PE transpose wins.)

DMA queues: w loads ride the gpsimd (SWDGE) queue, x loads + y stores the
sync queue, xbar transposes the scalar queue.
"""

import sys

for _p in ("/opt/trn_rl_repo", "/opt/pypackages"):
    if _p not in sys.path:
        sys.path.append(_p)

import numpy as np

import concourse.bacc as bacc
import concourse.mybir as mybir
from concourse.bass import _add_dep_helper
from concourse.bass_utils import run_bass_kernel_spmd
from concourse.masks import make_identity
from concourse.tile import TileContext

N_CORES = 8
B, S, IN_F, OUT_F = 4, 2048, 4096, 4096
GROUP = 128
M_FULL = B * S  # 8192


def build_program(M=M_FULL // N_CORES, K=IN_F, N=OUT_F, n_tile=512, ld=1024):
    """Emit the per-core Bass program (SPMD: same program on all cores)."""
    P = 128
    KT = K // P            # k chunks of 128 (contraction)
    MT = M // P            # m tiles of 128
    NT = N // n_tile       # out-feature tiles
    NSUB = n_tile // P     # 128-row o sub-blocks per o tile
    LC = K // ld           # load chunks per 128-row block
    LG = ld // P           # 128-col groups per load chunk
    G = K // GROUP         # scale groups along in_features
    NB = N // P            # o blocks of 128
    bf16 = mybir.dt.bfloat16
    f32 = mybir.dt.float32

    nc = bacc.Bacc("TRN2", target_bir_lowering=False, debug=False)
    x_d = nc.dram_tensor("x", [M, K], f32, kind="ExternalInput").ap()
    w_d = nc.dram_tensor("sw", [N, K], f32, kind="ExternalInput").ap()
    sc_d = nc.dram_tensor("sc", [N, G], f32, kind="ExternalInput").ap()
    y_d = nc.dram_tensor("y", [M, N], bf16, kind="ExternalOutput").ap()

    with TileContext(nc) as tc:
        with (
            tc.tile_pool(name="consts", bufs=1) as consts,
            tc.tile_pool(name="xt_pool", bufs=1) as xt_pool,
            tc.tile_pool(name="wt_pool", bufs=3) as wt_pool,
            tc.tile_pool(name="wload", bufs=3) as wload_pool,
            tc.tile_pool(name="wstage", bufs=6) as wstage_pool,
            tc.tile_pool(name="xload", bufs=2) as xload_pool,
            tc.tile_pool(name="xstage", bufs=2) as xstage_pool,
            tc.tile_pool(name="ysb", bufs=4) as y_pool,
            tc.tile_pool(name="acc", bufs=5, space="PSUM") as psum_a,
            tc.tile_pool(name="ptr", bufs=3, space="PSUM") as psum_t,
        ):
            ident = consts.tile([P, P], bf16)
            make_identity(nc, ident)

            # scales resident: sc_sb[p, ob*G + g] = scales[ob*128 + p, g]
            sc_sb = consts.tile([P, NB * G], f32)
            nc.sync.dma_start(
                out=sc_sb.rearrange("p (ob g) -> p ob g", ob=NB),
                in_=sc_d.rearrange("(ob p) g -> p ob g", p=P),
            )

            # work splitters: scale ops 3:1 DVE:ACT, casts/evacs alternate
            st = [0, 0]

            def veng_scale(out, in_, scalar):
                i = st[0]
                st[0] += 1
                if i % 4 < 3:
                    nc.vector.tensor_scalar_mul(out=out, in0=in_, scalar1=scalar)
                else:
                    nc.scalar.mul(out, in_, scalar)

            def veng_copy(out, in_):
                i = st[1]
                st[1] += 1
                if i % 2 == 0:
                    nc.vector.tensor_copy(out=out, in_=in_)
                else:
                    nc.scalar.copy(out=out, in_=in_)

            # ---- x^T resident (bf16): xT[p, k, m] ----
            xT = xt_pool.tile([P, KT, M], bf16)

            def emit_x_chunk(mt, lc):
                xin = xload_pool.tile([P, ld], f32, tag="xload")
                nc.sync.dma_start(
                    out=xin,
                    in_=x_d[mt * P:(mt + 1) * P, lc * ld:(lc + 1) * ld],
                )
                xst = xstage_pool.tile([P, ld], bf16, tag="xstage")
                veng_copy(xst, xin)
                # PE transpose against identity, 4 blocks per PSUM tile
                for h in range(LG // 4):
                    pt = psum_t.tile([P, 512], f32, tag="pt")
                    for g in range(4):
                        c = h * 4 + g
                        nc.tensor.matmul(
                            pt[:, g * P:(g + 1) * P],
                            xst[:, c * P:(c + 1) * P],
                            ident,
                            start=True,
                            stop=True,
                        )
                    pt_v = pt.rearrange("p (g c) -> p g c", g=4)
                    veng_copy(
                        xT[:, lc * LG + h * 4:lc * LG + h * 4 + 4,
                           mt * P:(mt + 1) * P],
                        pt_v,
                    )

            # ---- w_eff^T build for one 512-wide o tile (PE transposes) ----
            def emit_build_chunk(wT, ot, c):
                """One (j, lc) chunk of the wT build: load, scale, transpose."""
                j, lc = divmod(c, LC)
                ob = ot * NSUB + j
                win = wload_pool.tile([P, ld], f32, tag="wload")
                nc.gpsimd.dma_start(
                    out=win,
                    in_=w_d[ob * P:(ob + 1) * P, lc * ld:(lc + 1) * ld],
                )
                wst = wstage_pool.tile([P, ld], bf16, tag="wstage")
                for g in range(LG):
                    gk = lc * LG + g
                    veng_scale(
                        wst[:, g * P:(g + 1) * P],
                        win[:, g * P:(g + 1) * P],
                        sc_sb[:, ob * G + gk:ob * G + gk + 1],
                    )
                for h in range(LG // 4):
                    pt = psum_t.tile([P, 512], f32, tag="pt")
                    for g in range(4):
                        cc = h * 4 + g
                        nc.tensor.matmul(
                            pt[:, g * P:(g + 1) * P],
                            wst[:, cc * P:(cc + 1) * P],
                            ident,
                            start=True,
                            stop=True,
                        )
                    pt_v = pt.rearrange("p (g c) -> p g c", g=4)
                    veng_copy(
                        wT[:, lc * LG + h * 4:lc * LG + h * 4 + 4,
                           j * P:(j + 1) * P],
                        pt_v,
                    )

            NCH = NSUB * LC  # build chunks per o tile (16)

            def build_wT_all(ot):
                wT = wt_pool.tile([P, KT, n_tile], bf16, tag="wt")
                for c in range(NCH):
                    emit_build_chunk(wT, ot, c)
                return wT

            def emit_m_tile(ot, wT_v, mt):
                acc = psum_a.tile([P, n_tile], f32, tag="acc")
                for k in range(KT):
                    nc.tensor.matmul(
                        acc,
                        xT[:, k, mt * P:(mt + 1) * P],
                        wT_v[:, k],
                        start=(k == 0),
                        stop=(k == KT - 1),
                    )
                ysb = y_pool.tile([P, n_tile], bf16, tag="ysb")
                nc.vector.tensor_copy(out=ysb, in_=acc)
                nc.sync.dma_start(
                    out=y_d[mt * P:(mt + 1) * P,
                            ot * n_tile:(ot + 1) * n_tile],
                    in_=ysb,
                )

            # ---- prologue: first two wT builds, then x phase with o-tile 0
            #      matmuls folded in so the PE never idles while x loads ----
            wT_cur = build_wT_all(0)
            wT_nxt = build_wT_all(1) if NT > 1 else None
            wT_pre2 = (wt_pool.tile([P, KT, n_tile], bf16, tag="wt",
                                    name="wT_pre2")
                       if NT > 2 else None)
            for mt in range(MT):
                for lc in range(LC):
                    emit_x_chunk(mt, lc)
                emit_m_tile(0, wT_cur, mt)
                if wT_pre2 is not None:
                    for c in range(2 * mt, 2 * mt + 2):
                        emit_build_chunk(wT_pre2, 2, c)

            # ---- main loop over remaining o tiles, build for ot+2
            #      interleaved so DVE evacs never sit behind scale bursts ----
            wT_cur, wT_nxt = wT_nxt, wT_pre2
            for ot in range(1, NT):
                build = ot + 2 < NT
                wT_pre = (wt_pool.tile([P, KT, n_tile], bf16, tag="wt",
                                       name="wT_pre")
                          if build else None)
                for mt in range(MT):
                    emit_m_tile(ot, wT_cur, mt)
                    if build:
                        for c in range(2 * mt, 2 * mt + 2):
                            emit_build_chunk(wT_pre, ot + 2, c)
                wT_cur = wT_nxt
                wT_nxt = wT_pre

    nc.compile()
    return nc


_nc_cache = {}


def _get_nc(key, **kw):
    if key not in _nc_cache:
        _nc_cache[key] = build_program(**kw)
    return _nc_cache[key]


def _make_in_maps(x, sign_weights, scales):
    M_SH = M_FULL // N_CORES
    xf = np.ascontiguousarray(x.reshape(M_FULL, IN_F).astype(np.float32, copy=False))
    sw = np.ascontiguousarray(sign_weights.astype(np.float32, copy=False))
    sc = np.ascontiguousarray(scales.reshape(OUT_F, IN_F // GROUP))
    return [
        {"x": xf[c * M_SH:(c + 1) * M_SH], "sw": sw, "sc": sc}
        for c in range(N_CORES)
    ]


def _assemble(results):
    y = np.concatenate([results[c]["y"] for c in range(N_CORES)], axis=0)
    return y.reshape(B, S, OUT_F).astype(np.float32)


def kernel(x: np.ndarray, sign_weights: np.ndarray, scales: np.ndarray) -> np.ndarray:
    nc = _get_nc("full")
    in_maps = _make_in_maps(x, sign_weights, scales)
    res = run_bass_kernel_spmd(nc, in_maps, core_ids=list(range(N_CORES)))
    return _assemble(res.results)
